# revision 53
# baseline (speedup 1.0000x reference)
"""Trainium2 Bass kernel for the CandidateFinder sparse-attention problem.

Computes, for each (batch, query) row, the first K_MAX=64 key indices whose
32-bit sign pattern exactly matches the query's in either of two dim groups
(dims 0:32, 32:64), padded with -1.

Approach (per core; 8 cores = 4 batches x 2 query halves):
  - inputs arrive as bf16 (sign-preserving host cast, halves DMA bytes);
    signs scaled to +-0.5 ((x>0) - 0.5 via one DVE tensor_scalar pass per
    group-tensor, exact, 4x mode on bf16); per group
    S_g[q,j]/4 = sum_d q_d k_d is a quarter-integer in [-8,8];
    match <=> S_g/4 == 8. (TensorE, K=34.)
  - two extra contraction rows add ramp(j) = (2048-j)*2^-13 (sum of two
    bf16-exact products), so P_g = S_g/4 + ramp is exact in fp32 PSUM and
    strictly decreasing in j for fixed S.
  - per query tile, the key axis is processed in two PSUM phases:
      h0 (keys 0:1024): both groups' matmuls land in one 4-bank PSUM tile;
      ONE 2048-wide ACT relu(P - 8) evacuates it to fp16 (matched
      positions give exactly (2048-j)*2^-13, rest 0), and its accumulator
      gives an exact-unless-2^-64 upper bound on the merged sum; a DVE
      all-fp16 tensor_tensor max (2x mode) merges the two groups.
      h1 (keys 1024:2048): the groups use separate PSUM tiles so their
      evacuations free them independently (PSUM dependencies are tracked
      per tile): ACT relus group 2 while DVE evacuates group 1 fused with
      the merge (scalar_tensor_tensor, accum_out = exact merged sum).
    ACT and DVE run near-balanced (~3.1us each per tile); PE warmup
    matmuls at t~0 lift the HAM clock gate, and a dependency-free ACT
    warmup pulls the ~1.3us activation-table load off the critical path.
  - three pairwise fp16 max folds shrink the row 2048 -> 256 before the
    DVE `max` (hardware top-8, descending) extracts the first <=8
    matching j; max never alters values, so survivors encode j exactly.
  - loss detection by sum conservation, batched per 4-tile half:
    sum(top8) == acc(h0) + acc(h1) iff no fold collision dropped a match
    and the row had <= 8 matches; flag = (sum > top8sum) forces a
    positive 8th slot, which triggers the exact host fallback.
  - three 2-source ops per half decode the top-8 values to j / -1 into a
    packed [128, 4*64] half of the output block; each half is written back
    with its own DMA so the first overlaps the second half's compute.
  - rows whose 8th candidate decodes as a real match (>=8 real matches, or
    the collision flag) are recomputed exactly on the host with numpy. With
    random normal inputs this never triggers: a match needs a 2^-32
    sign-pattern collision.

Self-contained: hardcodes shapes from the problem spec.
"""

import numpy as np

B = 4
L = 2048
D = 64
K_MAX = 64
N_CORES = 8
QSH = B * L // N_CORES  # 1024 queries per core
N_QT = QSH // 128       # 8 query tiles per core

_CACHE = {}


def _build_program(reps=1):
    from contextlib import ExitStack

    import concourse.bacc as bacc
    import concourse.mybir as mybir
    import concourse.tile as tile

    dt = mybir.dt
    Alu = mybir.AluOpType

    # Bacc (not raw Bass): its legalization passes split multi-sem waits,
    # which PE instructions can't carry (1 wait max per instruction).
    nc = bacc.Bacc("TRN2", target_bir_lowering=False, debug=False)
    # bf16 inputs: rounding fp32 -> bf16 preserves the sign bit, and bf16
    # flushes to 0.0 only below 1e-40, unreachable for randn fp32 data, so
    # (x > 0) is unchanged. Halves DMA bytes and lets the sign ops run in
    # the DVE's 4x perf mode.
    qT_d = nc.declare_dram_parameter("qT", [D, QSH], dt.bfloat16, isOutput=False)
    kT_d = nc.declare_dram_parameter("kT", [D, L], dt.bfloat16, isOutput=False)
    ramp_d = nc.declare_dram_parameter("ramp", [2, L], dt.bfloat16, isOutput=False)
    # packed output: out[p, 64*t + c] = candidate c of query row t*128 + p
    out_d = nc.declare_dram_parameter("out", [128, N_QT * K_MAX], dt.int32,
                                      isOutput=True)

    with tile.TileContext(nc) as tc, ExitStack() as ctx:
        consts = ctx.enter_context(tc.tile_pool(name="consts", bufs=1))
        vals = ctx.enter_context(tc.tile_pool(name="vals", bufs=3))
        outs = ctx.enter_context(tc.tile_pool(name="outs", bufs=2))
        psum = ctx.enter_context(tc.tile_pool(name="psum", bufs=1, space="PSUM"))

        # ---- load raw (transposed) inputs, split by dim group ----
        # all on SP (the ACT queue would stall a DMA setup behind the
        # activation table load); group 1 first — it gates the first signs
        qraw = consts.tile([D, QSH], dt.bfloat16)
        kraw = consts.tile([D, L], dt.bfloat16)
        nc.sync.dma_start(kraw[0:32, :], kT_d[0:32, :])
        nc.sync.dma_start(qraw[0:32, :], qT_d[0:32, :])
        nc.sync.dma_start(kraw[32:64, :], kT_d[32:64, :])
        nc.sync.dma_start(qraw[32:64, :], qT_d[32:64, :])

        # constants: all memsets/small DMAs on the otherwise-idle Pool
        # engine, ordered by when they are first needed (z64 first: it
        # feeds the PE warmup; the ramp DMAs gate the first matmuls so
        # they precede the slower ones-row memsets)
        z64 = consts.tile([128, 64], dt.float32, tag="z64")
        nc.gpsimd.memset(z64[:], 0.0)
        bias8 = consts.tile([128, 1], dt.float32, tag="bias8")
        nc.gpsimd.memset(bias8[:], -8.0)

        # ACT warmup: a dependency-free activation so the ~1.3us Relu
        # table load happens at t~0 instead of attaching itself (and its
        # successor's waits) to the first real evacuation.
        actw = consts.tile([128, 1], dt.float32, tag="actw")
        nc.scalar.activation(
            actw[:], z64[:, 0:1], mybir.ActivationFunctionType.Relu,
            bias=0.0, scale=1.0)

        # ---- sign tiles (+ ramp rows) ----
        # QS[g]: [34, QSH]  rows 0:32 = query signs (+-0.5), rows 32/33 = 1.0
        # KS[g]: [34, L]    rows 0:32 = key signs (+-0.5), rows 32/33 = ramp
        # All four sign passes on DVE: bf16 in/out, SBUF, step 1 -> 4x mode.
        # (x>0) - 0.5 -> +-0.5 exactly; x == 0 -> -0.5 like the reference.
        QS = []
        KS = []
        for g in range(2):
            qs = consts.tile([34, QSH], dt.bfloat16, tag=f"qs{g}")
            ks = consts.tile([34, L], dt.bfloat16, tag=f"ks{g}")
            QS.append(qs)
            KS.append(ks)
            # ramp terms (host-precomputed bf16 constants) into rows 32/33
            nc.gpsimd.dma_start(ks[32:34, :], ramp_d[:])
        for g in range(2):
            nc.gpsimd.memset(QS[g][32:34, :], 1.0)
        # key signs first: they are the longer pass and gate the first
        # matmul together with the query signs
        for g in range(2):
            lo, hi = g * 32, (g + 1) * 32
            nc.vector.tensor_scalar(
                out=KS[g][0:32, :], in0=kraw[lo:hi, :],
                scalar1=0.0, scalar2=0.5, op0=Alu.is_gt, op1=Alu.subtract)
            nc.vector.tensor_scalar(
                out=QS[g][0:32, :], in0=qraw[lo:hi, :],
                scalar1=0.0, scalar2=0.5, op0=Alu.is_gt, op1=Alu.subtract)

        c2048 = consts.tile([128, 64], dt.float32, tag="c2048")
        nc.gpsimd.memset(c2048[:], 2048.0)
        b2047 = consts.tile([128, 1], dt.float32, tag="b2047")
        nc.gpsimd.memset(b2047[:], -2047.0)
        # all 8 query tiles' top-8 values, decoded per 4-tile half
        t8all = consts.tile([128, 64], dt.float16, tag="t8all")
        # per-tile exact sums: col 2t = Pool accum of the merged h0 block,
        # col 2t+1 = DVE accum of the merged h1 block (both exact: a
        # both-group match contributes once, like the reference's union)
        # (+2 spare cols: tile 0 splits its h0 evacuation for an earlier
        # pipeline start, so its h0 sum arrives as two accums)
        svall = consts.tile([128, 2 * N_QT + 2], dt.float32, tag="svall")
        # packed output block; -1 everywhere the decode doesn't overwrite
        o2 = consts.tile([128, N_QT * K_MAX], dt.int32, tag="o2")
        nc.gpsimd.memset(o2[:], -1)

        # ---- PE warmup: ~10 zero matmuls so the HAM clock gate reaches
        # full speed before the first real matmuls (the activity window is
        # ~3.4us; these run while the DMAs and signs are still in flight).
        # z64 is all zeros so the scratch PSUM results are finite; they
        # reuse the p01 storage, which tile 0 overwrites with start=True.
        pwarm = psum.tile([128, 1024], dt.float32, tag="p0b")
        for w in range(10):
            nc.tensor.matmul(
                pwarm[0:64, w * 64:w * 64 + 64],
                z64[:], z64[:, 0:64], start=True, stop=True)

        t8v = t8all.rearrange("p (t c) -> p t c", c=8)
        sv2 = svall[:, 0:2 * N_QT].rearrange("p (t c) -> p t c", c=2)
        o2v = o2.rearrange("p (t c) -> p t c", c=K_MAX)

        def tail(t0, nt, extra_accum=False, decode_on_act=False):
            """Flags + decode + writeback for tiles t0..t0+nt-1.

            decode_on_act: run the two single-source decode stages on ACT
            (its slack) instead of the saturated DVE — used for the
            mid-body tails, whose DMAs are off the critical path; the
            final tail keeps them on DVE to avoid cross-engine latency
            in the closing chain.
            """
            ts = slice(t0, t0 + nt)
            # sum(val) from the two exact merged-block accums
            up4 = outs.tile([128, 4], dt.float32, tag="up4")
            nc.vector.tensor_tensor(
                out=up4[:, 0:nt], in0=sv2[:, ts, 0], in1=sv2[:, ts, 1],
                op=Alu.add)
            if extra_accum:
                # tile 0's second h0 accum (split evacuation)
                nc.vector.tensor_tensor(
                    out=up4[:, 0:1], in0=up4[:, 0:1],
                    in1=svall[:, 2 * N_QT:2 * N_QT + 1], op=Alu.add)
            # sum of the extracted top-8 values (exact in fp32)
            ts4 = outs.tile([128, 4], dt.float32, tag="ts4")
            nc.vector.tensor_reduce(
                out=ts4[:, 0:nt], in_=t8v[:, ts, :], axis=mybir.AxisListType.X,
                op=Alu.add)
            # flag = some match was dropped (fold collision, > 8 matches,
            # or a 2^-64 both-group match). Written straight into column 63
            # of each tile's output block (the device otherwise always
            # leaves it -1): the host reads it there, so the decode below
            # no longer depends on the flag chain and runs in parallel.
            nc.vector.tensor_tensor(
                out=o2v[:, ts, 63], in0=up4[:, 0:nt], in1=ts4[:, 0:nt],
                op=Alu.is_gt)
            # decode: matched v = (2048-j)*2^-13 => u = 2048 - 8192*v = j;
            # unmatched v = 0 => u = 2048 -> -1.
            w = 8 * nt
            cols = slice(8 * t0, 8 * t0 + w)
            u = outs.tile([128, 32], dt.float32, tag="u")
            pad = outs.tile([128, 32], dt.float32, tag="pad")
            if decode_on_act:
                nc.scalar.activation(
                    u[:, 0:w], t8all[:, cols],
                    mybir.ActivationFunctionType.Copy,
                    bias=2048.0, scale=-8192.0)
                nc.scalar.activation(
                    pad[:, 0:w], u[:, 0:w],
                    mybir.ActivationFunctionType.Relu,
                    bias=b2047[:], scale=1.0)
            else:
                nc.vector.scalar_tensor_tensor(
                    out=u[:, 0:w], in0=t8all[:, cols], scalar=-8192.0,
                    in1=c2048[:, 0:w], op0=Alu.mult, op1=Alu.add)
                nc.vector.scalar_tensor_tensor(
                    out=pad[:, 0:w], in0=u[:, 0:w], scalar=-2047.0,
                    in1=z64[:, 0:w], op0=Alu.add, op1=Alu.max)
            # o = u - 2049*pad -> j or -1 (int32 cast on write), scattered
            # into the first 8 columns of each tile's 64-column block
            nc.vector.scalar_tensor_tensor(
                out=o2v[:, ts, 0:8],
                in0=pad[:, 0:w].rearrange("p (t c) -> p t c", c=8),
                scalar=-2049.0,
                in1=u[:, 0:w].rearrange("p (t c) -> p t c", c=8),
                op0=Alu.mult, op1=Alu.add)
            # writeback from SP (idle by now; cheaper DGE setup than SWDGE)
            nc.sync.dma_start(out_d[:, 64 * t0:64 * (t0 + nt)],
                              o2[:, 64 * t0:64 * (t0 + nt)])

        # ---- main loop over query tiles ----
        # reps>1 repeats the whole body inside one NEFF (timing only).
        for _ in range(reps):
            for t in range(N_QT):
                val = vals.tile([128, 1024], dt.float16, tag="val")
                vh0 = vals.tile([128, 2048], dt.float16, tag="vh0")
                v2h1 = vals.tile([128, 1024], dt.float16, tag="v2h1")

                # half 0: both groups' matmuls land in one 4-bank PSUM tile
                # so ONE 2048-wide ACT relu evacuates both; its accumulator
                # gives sum(vh0) >= sum(val_h0), tight unless a 2^-64
                # both-group match (which then just false-positives the
                # exact host fallback). DVE merges with one all-fp16 2x tt.
                p01 = psum.tile([128, 2048], dt.float32, tag="p01")
                for g in range(2):
                    for n in range(2):
                        nc.tensor.matmul(
                            p01[:, g * 1024 + n * 512:g * 1024 + (n + 1) * 512],
                            QS[g][:, t * 128:(t + 1) * 128],
                            KS[g][:, n * 512:(n + 1) * 512],
                            start=True, stop=True)
                if t == 0:
                    # tile 0 only: split the wide evacuation so ACT starts
                    # right after the g1 matmuls instead of waiting for all
                    # four chunks (the two accums are summed in the tail)
                    nc.scalar.activation(
                        vh0[:, 0:1024], p01[:, 0:1024],
                        mybir.ActivationFunctionType.Relu,
                        bias=bias8[:], scale=1.0,
                        accum_out=sv2[:, t, 0:1])
                    nc.scalar.activation(
                        vh0[:, 1024:2048], p01[:, 1024:2048],
                        mybir.ActivationFunctionType.Relu,
                        bias=bias8[:], scale=1.0,
                        accum_out=svall[:, 2 * N_QT:2 * N_QT + 1])
                else:
                    nc.scalar.activation(
                        vh0[:], p01[:], mybir.ActivationFunctionType.Relu,
                        bias=bias8[:], scale=1.0,
                        accum_out=sv2[:, t, 0:1])
                nc.vector.tensor_tensor(
                    out=val[:, 0:1024], in0=vh0[:, 0:1024],
                    in1=vh0[:, 1024:2048], op=Alu.max)

                # half 1: ACT evacuates group 2 (own PSUM tile), DVE
                # evacuates group 1 fused with the merge (PSUM source, 1x)
                # + exact accum; separate tiles keep the two loops
                # independent.
                p0b = psum.tile([128, 1024], dt.float32, tag="p0b")
                p1b = psum.tile([128, 1024], dt.float32, tag="p1b")
                for g, pg in ((1, p1b), (0, p0b)):
                    for n in range(2):
                        nc.tensor.matmul(
                            pg[:, n * 512:(n + 1) * 512],
                            QS[g][:, t * 128:(t + 1) * 128],
                            KS[g][:, 1024 + n * 512:1024 + (n + 1) * 512],
                            start=True, stop=True)
                nc.scalar.activation(
                    v2h1[:], p1b[:], mybir.ActivationFunctionType.Relu,
                    bias=bias8[:], scale=1.0)
                val1 = vals.tile([128, 1024], dt.float16, tag="val1")
                nc.vector.scalar_tensor_tensor(
                    out=val1[:], in0=p0b[:], scalar=-8.0,
                    in1=v2h1[:], op0=Alu.add, op1=Alu.max,
                    accum_out=sv2[:, t, 1:2])

                # fold 2048 -> 256 (all-fp16 2x tt) and extract the top-8
                m1 = vals.tile([128, 1024], dt.float16, tag="m1")
                nc.vector.tensor_tensor(
                    out=m1[:], in0=val[:, 0:1024], in1=val1[:], op=Alu.max)
                m2 = vals.tile([128, 512], dt.float16, tag="m2")
                nc.vector.tensor_tensor(
                    out=m2[:], in0=m1[:, 0:512], in1=m1[:, 512:1024],
                    op=Alu.max)
                m3 = vals.tile([128, 256], dt.float16, tag="m3")
                nc.vector.tensor_tensor(
                    out=m3[:], in0=m2[:, 0:256], in1=m2[:, 256:512],
                    op=Alu.max)
                nc.vector.max(t8all[:, 8 * t:8 * t + 8], m3[:])

                if t == 3:
                    tail(0, 4, extra_accum=True, decode_on_act=True)
                elif t == 6:
                    tail(4, 3, decode_on_act=True)
            tail(7, 1)

    return nc


def _get_program():
    if "prog" not in _CACHE:
        nc = _build_program()
        if not nc.is_finalized():
            nc.finalize()  # Bacc: runs wait-splitting + reg-alloc passes
        _CACHE["prog"] = nc
    return _CACHE["prog"]


def _ramp_rows():
    """[2, L] bf16 rows summing (via the all-ones weight rows) to
    ramp(j) = (2048-j)*2^-13: hi = (128-(j>>4))*2^-9, lo = -(j&15)*2^-13.
    Every term is exactly representable in bf16, and relu(P-16) lands in
    (0, 0.25] where fp16 spacing is <= 2^-13, so values stay exact."""
    import ml_dtypes
    j = np.arange(L)
    hi = (128 - (j >> 4)).astype(np.float32) * 2.0 ** -9
    lo = -(j & 15).astype(np.float32) * 2.0 ** -13
    return np.stack([hi, lo]).astype(ml_dtypes.bfloat16)


def _make_in_maps(q, k):
    import ml_dtypes
    ramp = _ramp_rows()
    in_maps = []
    for c in range(N_CORES):
        b, h = divmod(c, 2)
        # bf16 rounding preserves (x > 0) for all reachable randn fp32
        qT = np.ascontiguousarray(
            q[b, h * QSH:(h + 1) * QSH, :].T.astype(ml_dtypes.bfloat16))
        kT = np.ascontiguousarray(k[b].T.astype(ml_dtypes.bfloat16))
        in_maps.append({"qT": qT, "kT": kT, "ramp": ramp})
    return in_maps


def run_device(q, k, trace=False):
    """Run the bass kernel on the 8 cores.

    Returns (full_out, any_loss_flag): column 63 of each device block
    carries the per-row sum-conservation flag (1 = a match was dropped by
    a fold collision / >8 matches / 2^-64 both-group match); it is read
    out and restored to the -1 padding the reference expects.
    """
    from concourse.bass_utils import run_bass_kernel_spmd

    res = run_bass_kernel_spmd(
        _get_program(), _make_in_maps(q, k), list(range(N_CORES)), trace=trace)
    full = np.empty((B, L, K_MAX), np.int32)
    for c in range(N_CORES):
        b, h = divmod(c, 2)
        # out[p, 64t+c] = result for query row t*128+p
        blk = res.results[c]["out"].reshape(128, N_QT, K_MAX)
        full[b, h * QSH:(h + 1) * QSH, :] = (
            blk.transpose(1, 0, 2).reshape(QSH, K_MAX))
    flagged = bool((full[..., 63] == 1).any())
    full[..., 63] = -1
    return full, flagged


def _reference_numpy(q, k):
    """Exact numpy fallback (used only if some row has >= 8 matches)."""
    out = np.full((B, L, K_MAX), -1, np.int32)
    for b in range(B):
        qb = (q[b] > 0)
        kb = (k[b] > 0)
        match = np.zeros((L, L), bool)
        for lo in (0, 32):
            qg = qb[:, lo:lo + 32]
            kg = kb[:, lo:lo + 32]
            # pack 32 bits into one uint32 per row for exact equality
            qc = np.packbits(qg, axis=1).view(">u4").ravel()
            kc = np.packbits(kg, axis=1).view(">u4").ravel()
            match |= qc[:, None] == kc[None, :]
        for i in range(L):
            idx = np.nonzero(match[i])[0][:K_MAX]
            out[b, i, :len(idx)] = idx
    return out


def kernel(query_up, key_up, head_idx=None, **_unused):
    q = np.asarray(query_up, dtype=np.float32)
    k = np.asarray(key_up, dtype=np.float32)
    assert q.shape == (B, L, D) and k.shape == (B, L, D)
    full, flagged = run_device(q, k)
    # Exact overflow detection: a non(-1) 8th candidate means the row had
    # >= 8 matches (candidates 9.. might have been dropped); the device
    # flag covers fold collisions below that threshold.
    if flagged or (full[..., 7] != -1).any():
        full = _reference_numpy(q, k)
    return full
